# revision 5
# baseline (speedup 1.0000x reference)
"""Trainium2 Bass kernel for MixerDiffAttention (differential attention).

Sharding: tensor-parallel over the 8 (n_head//2) head groups across 8 cores
(data-parallel over B is trivial since B=1). Each core computes the QKV
projections for its head group, both differential attention branches, the
normalized combination y1 - lambda*y2, and its head's partial product with
the row-sharded c_proj. The host sums the 8 partial outputs (the unshard
step for row-parallel tensor parallelism).

v3 layout notes (per core, head h):
  - QKV projections x-stationary: out[t, (q1 q2 k1 k2 v)] per 128-row t-block,
    accumulated over 8 contraction chunks.
  - Rotary is applied to RAW q/k, then the rms_norm scale is applied to the
    rotated values (the rotation is orthogonal per (i, i+32) pair, so it
    commutes with the per-subhead scalar normalization). cos/sin tables are
    pre-expanded over the 4 subheads on the host so every rotary DVE op is a
    dense step-1 bf16 op (2x mode).
  - 1/sqrt(msq+eps) = exp(-0.5*ln(msq+eps)) computed ONCE for all 16 t-blocks
    after stage B, so ACT loads the natural_log table set once and the exp
    set once -- no per-group table thrash (Rsqrt activation is banned).
  - schedule: B0..B3 (QKV+rotary) -> rsc -> S0 C0 S1 C1 S2 C2 S3 C3, where
    S_c scales the rotated q/k group and transposes it to [c, t]: q via PE
    transposes (keeps the PE stream dense), k via the DMA xbar on the two
    then-idle HWDGE queues.
  - scores for the two branches are written into one 2-bank PSUM tile and
    exp'd in a single ACT op; the two score matmuls have K=64 at partition
    bases 0/64 so they run concurrently in the PE array (row tiling).
  - exp never overflows: q/k are rms-normalized so |score*scale| <= 8.
  - PSUM budget is exactly 8 banks: tag pp 2x2 (QKV groups / score pairs /
    proj outputs / k-stage), py 2 (PV accum + q-transposes), pd 2 (denom
    accum + q-transposes).
"""

import os
import sys

import numpy as np

for _p in ("/opt/trn_rl_repo", "/root/.axon_site/_ro/trn_rl_repo"):
    if os.path.isdir(_p) and _p not in sys.path:
        sys.path.insert(0, _p)

import ml_dtypes

import concourse.bass as bass
import concourse.mybir as mybir
import concourse.tile as tile
from concourse import bacc
from concourse.bass import ds, ts
from concourse.bass_utils import run_bass_kernel_spmd
from concourse.masks import make_identity

BF16 = mybir.dt.bfloat16
F32 = mybir.dt.float32
AF = mybir.ActivationFunctionType
ALU = mybir.AluOpType

N_HEAD = 16
D = 1024
HD = 64  # head dim
T = 2048
NCORES = 8
TB = T // 128  # 16 t-blocks
KC = D // 128  # 8 contraction chunks
NTC = T // 512  # 4 t-chunks of 512
LAMBDA_INIT = 0.8 - 0.6 * float(np.exp(-0.3 * 1))
EPS = float(np.finfo(np.float32).eps)
SCALE = 1.0 / 8.0  # 1/sqrt(64)

_CACHE = {}


def _build_program(lam: float) -> bass.Bass:
    nc = bacc.Bacc("TRN2", target_bir_lowering=False, debug=False)

    xT = nc.declare_dram_parameter("xT", [D, T], BF16, isOutput=False)
    wqkv = nc.declare_dram_parameter("wqkv", [D, 384], BF16, isOutput=False)
    wpp = nc.declare_dram_parameter("wpp", [128, D], BF16, isOutput=False)
    cos_d = nc.declare_dram_parameter("cos", [128, TB * 128], BF16, isOutput=False)
    sin_d = nc.declare_dram_parameter("sin", [128, TB * 128], BF16, isOutput=False)
    diag_d = nc.declare_dram_parameter("diag", [128, 256], BF16, isOutput=False)
    outTp = nc.declare_dram_parameter("outTp", [D, T], BF16, isOutput=True)

    with tile.TileContext(nc) as tc:
        with (
            tc.tile_pool(name="const", bufs=1) as cpool,
            tc.tile_pool(name="work", bufs=3) as wpool,
            tc.tile_pool(name="ptile", bufs=3) as ppool,
            tc.tile_pool(name="ostage", bufs=3) as opool,
            tc.tile_pool(name="psum", bufs=1, space="PSUM") as psum_pool,
        ):
            # ---- persistent SBUF tensors ----
            xT_sb = cpool.tile([128, KC, T], BF16, tag="xT")
            wqkv_sb = cpool.tile([128, KC, 384], BF16, tag="wqkv")
            wpp_sb = cpool.tile([128, KC, 128], BF16, tag="wpp")
            cos_sb = cpool.tile([128, TB, 4, 32], BF16, tag="cos")
            sin_sb = cpool.tile([128, TB, 4, 32], BF16, tag="sin")
            diag_sb = cpool.tile([128, 2, 128], BF16, tag="diag")
            ones_sb = cpool.tile([128, 128], BF16, tag="ones")
            ident_sb = cpool.tile([128, 128], BF16, tag="ident")
            qk_sb = cpool.tile([128, TB, 4, HD], BF16, tag="qk")  # raw q1 q2 k1 k2
            rot_sb = cpool.tile([128, TB, 4, HD], BF16, tag="rot")  # rotated, unscaled
            ssq_sb = cpool.tile([128, TB, 4], F32, tag="ssq")
            rsc_sb = cpool.tile([128, TB, 4], F32, tag="rsc")
            qT_sb = cpool.tile([128, T], BF16, tag="qT")  # rows 0:64 g0, 64:128 g1
            kT_sb = cpool.tile([128, T], BF16, tag="kT")
            v_sb = cpool.tile([128, TB, 128], BF16, tag="v")  # [s-part, tb, j]

            # ---- load constants ----
            # wqkv (sync) + x t-chunk 0 (gpsimd/scalar) gate the first QKV
            # matmuls; everything else streams in behind them.
            for kc in range(KC):
                nc.sync.dma_start(out=wqkv_sb[:, kc, :], in_=wqkv[ts(kc, 128), :])
            for kc in range(KC):
                eng = nc.gpsimd if kc % 2 == 0 else nc.scalar
                eng.dma_start(
                    out=xT_sb[:, kc, ts(0, 512)], in_=xT[ts(kc, 128), ts(0, 512)]
                )
            nc.scalar.dma_start(
                out=cos_sb[:].rearrange("p a b c -> p (a b c)"), in_=cos_d[:, :]
            )
            nc.scalar.dma_start(
                out=sin_sb[:].rearrange("p a b c -> p (a b c)"), in_=sin_d[:, :]
            )
            nc.gpsimd.dma_start(
                out=diag_sb[:].rearrange("p a b -> p (a b)"), in_=diag_d[:, :]
            )
            for tc_i in range(1, NTC):
                for kc in range(KC):
                    eng = (nc.sync, nc.gpsimd, nc.scalar)[kc % 3]
                    eng.dma_start(
                        out=xT_sb[:, kc, ts(tc_i, 512)],
                        in_=xT[ts(kc, 128), ts(tc_i, 512)],
                    )
            for kc in range(KC):
                nc.gpsimd.dma_start(out=wpp_sb[:, kc, :], in_=wpp[:, ts(kc, 128)])
            nc.vector.memset(ones_sb[:], 1.0)
            make_identity(nc, ident_sb[:])
            eps_sb = cpool.tile([128, 1], F32, tag="eps")
            nc.vector.memset(eps_sb[:], EPS)

            # ---- stage B: QKV projection + rotary for one group of 4 t-blocks
            def stage_b(c):
                for i in range(4):
                    tb = 4 * c + i
                    pqkv = psum_pool.tile(
                        [128, 2, 512], F32, tag="pp", bufs=2, name="pqkv"
                    )
                    qkv = pqkv[:].rearrange("p a b -> p (a b)")
                    for kc in range(KC):
                        nc.tensor.matmul(
                            qkv[:, 0:384],
                            xT_sb[:, kc, ts(tb, 128)],
                            wqkv_sb[:, kc, :],
                            start=(kc == 0),
                            stop=(kc == KC - 1),
                        )
                    # evacuate: v + raw qk (DVE casts)
                    nc.vector.tensor_copy(v_sb[:, tb, :], qkv[:, 256:384])
                    nc.vector.tensor_copy(
                        qk_sb[:, tb].rearrange("p a b -> p (a b)"), qkv[:, 0:256]
                    )

                # sum of squares per 64-wide subhead, for the whole group
                sq = wpool.tile([128, 4, 4, HD], BF16, tag="sq")
                qkg = qk_sb[:, ds(4 * c, 4)]
                nc.vector.tensor_mul(sq[:], qkg, qkg)
                nc.vector.reduce_sum(
                    ssq_sb[:, ds(4 * c, 4)], sq[:], axis=mybir.AxisListType.X
                )

                # rotary on raw qk: out1 = n1*cos + n2*sin ; out2 = n2*cos - n1*sin
                n1 = qkg[:, :, :, 0:32]
                n2 = qkg[:, :, :, 32:64]
                cosb = cos_sb[:, ds(4 * c, 4)]
                sinb = sin_sb[:, ds(4 * c, 4)]
                rotg = rot_sb[:, ds(4 * c, 4)]
                t1 = wpool.tile([128, 4, 4, 32], BF16, tag="t1")
                t2 = wpool.tile([128, 4, 4, 32], BF16, tag="t2")
                nc.vector.tensor_mul(t1[:], n1, cosb)
                nc.vector.tensor_mul(t2[:], n2, sinb)
                nc.vector.tensor_add(rotg[:, :, :, 0:32], t1[:], t2[:])
                nc.vector.tensor_mul(t1[:], n2, cosb)
                nc.vector.tensor_mul(t2[:], n1, sinb)
                nc.vector.tensor_sub(rotg[:, :, :, 32:64], t1[:], t2[:])

            # ---- stage S: apply rms scale to one group, transpose to [c, t]
            def stage_s(c):
                qkr = wpool.tile([128, 4, 4, HD], BF16, tag="qkr")
                rscb = (
                    rsc_sb[:, ds(4 * c, 4)].unsqueeze(3).broadcast_to([128, 4, 4, HD])
                )
                nc.vector.tensor_mul(qkr[:], rot_sb[:, ds(4 * c, 4)], rscb)
                for i in range(4):
                    tb = 4 * c + i
                    qblk = qkr[:, i, 0:2, :].rearrange("p a b -> p (a b)")
                    kblk = qkr[:, i, 2:4, :].rearrange("p a b -> p (a b)")
                    # q: PE transpose (keeps the PE warm), evacuate on DVE
                    ptq = psum_pool.tile(
                        [128, 128], BF16, tag=("py" if i % 2 == 0 else "pd"),
                        name="ptq",
                    )
                    nc.tensor.transpose(ptq[:], qblk, ident_sb[:])
                    nc.vector.tensor_copy(qT_sb[:, ts(tb, 128)], ptq[:])
                    # k: DMA xbar transpose on the idle HWDGE queues
                    eng = nc.sync if i % 2 == 0 else nc.scalar
                    eng.dma_start(
                        out=kT_sb[:, ts(tb, 128)], in_=kblk, transpose=True
                    )

            # ---- stage C: differential attention + partial projection for
            # one 512-wide t-chunk ----
            def stage_c(tc_i):
                nsb = 4 * tc_i + 4  # s-blocks touching this t-chunk
                py = psum_pool.tile([128, 2, 512], F32, tag="py", name="py")
                pd = psum_pool.tile([128, 2, 512], F32, tag="pd", name="pd")
                for si in range(nsb):
                    col0 = max(0, si * 128 - tc_i * 512)
                    w = 512 - col0
                    pp = psum_pool.tile(
                        [128, 2, 512], F32, tag="pp", bufs=2, name="pp"
                    )
                    for g in range(2):
                        nc.tensor.matmul(
                            pp[:, g, col0:512],
                            kT_sb[ds(g * 64, 64), ts(si, 128)],
                            qT_sb[ds(g * 64, 64), ds(tc_i * 512 + col0, w)],
                            start=True,
                            stop=True,
                        )
                    pt = ppool.tile([128, 2, 512], BF16, tag="pt")
                    nc.scalar.activation(
                        pt[:, :, col0:512], pp[:, :, col0:512], AF.Exp, scale=SCALE
                    )
                    if col0 > 0 or si * 128 == tc_i * 512:
                        # diagonal block: zero out s > t inside it
                        nc.vector.tensor_mul(
                            pt[:, :, col0 : col0 + 128],
                            pt[:, :, col0 : col0 + 128],
                            diag_sb[:],
                        )
                    for g in range(2):
                        nc.tensor.matmul(
                            py[:, g, col0:512],
                            v_sb[:, si, :],
                            pt[:, g, col0:512],
                            start=(si == 0),
                            stop=(si == nsb - 1),
                        )
                    for g in range(2):
                        nc.tensor.matmul(
                            pd[:, g, col0:512],
                            ones_sb[:],
                            pt[:, g, col0:512],
                            start=(si == 0),
                            stop=(si == nsb - 1),
                        )

                # normalize + combine the two branches
                rec = wpool.tile([128, 2, 512], F32, tag="rec")
                nc.vector.reciprocal_approx_fast(rec[:], pd[:])
                yn = wpool.tile([128, 2, 512], BF16, tag="yn")
                nc.vector.tensor_mul(yn[:], py[:], rec[:])
                ycomb = wpool.tile([128, 512], BF16, tag="ycomb")
                nc.vector.scalar_tensor_tensor(
                    ycomb[:], yn[:, 1], -lam, yn[:, 0], ALU.mult, ALU.add
                )

                # partial projection for this t-chunk
                for icp in range(KC // 2):
                    po = psum_pool.tile([128, 2, 512], F32, tag="pp", bufs=2, name="po")
                    for j in range(2):
                        ic = icp * 2 + j
                        nc.tensor.matmul(
                            po[:, j, :],
                            wpp_sb[:, ic, :],
                            ycomb[:],
                            start=True,
                            stop=True,
                        )
                    ost = opool.tile([128, 2, 512], BF16, tag="ost")
                    if icp % 2 == 0:
                        nc.vector.tensor_copy(ost[:], po[:])
                    else:
                        nc.scalar.copy(ost[:], po[:])
                    for j in range(2):
                        ic = icp * 2 + j
                        nc.sync.dma_start(
                            out=outTp[ts(ic, 128), ts(tc_i, 512)], in_=ost[:, j]
                        )

            # ---- emission schedule ----
            for c in range(4):
                stage_b(c)
            # rsc = 1/sqrt(ssq/64 + eps) = exp(-0.5*ln(ssq/64 + eps)), one op
            # pair for all 16 t-blocks (2 ACT table loads for the whole kernel)
            lssq = wpool.tile([128, TB, 4], F32, tag="lssq")
            nc.scalar.activation(
                lssq[:], ssq_sb[:], AF.Ln, bias=eps_sb[:], scale=1.0 / HD
            )
            nc.scalar.activation(rsc_sb[:], lssq[:], AF.Exp, scale=-0.5)
            for c in range(4):
                stage_s(c)
                stage_c(c)

    nc.compile()
    return nc


def _make_in_maps(x, Wq, Wk, Wv, Wproj):
    bf = ml_dtypes.bfloat16
    xT = np.ascontiguousarray(x[0].T).astype(bf)  # [D, T]

    # rotary tables: [tp, tb, 32] expanded over the 4 subheads -> [tp, tb, 4, 32]
    inv = 1.0 / (10000.0 ** (np.arange(0, HD, 2, dtype=np.float32) / HD))
    fr = np.outer(np.arange(T, dtype=np.float32), inv)  # [T, 32]
    cos = np.cos(fr).reshape(TB, 128, 32).transpose(1, 0, 2)  # [128, TB, 32]
    cos4 = np.broadcast_to(cos[:, :, None, :], (128, TB, 4, 32)).reshape(128, -1)
    sin = np.sin(fr).reshape(TB, 128, 32).transpose(1, 0, 2)
    sin4 = np.broadcast_to(sin[:, :, None, :], (128, TB, 4, 32)).reshape(128, -1)
    cos4, sin4 = np.ascontiguousarray(cos4).astype(bf), np.ascontiguousarray(
        sin4
    ).astype(bf)
    diag = np.triu(np.ones((128, 128), np.float32)).astype(bf)
    diag2 = np.ascontiguousarray(np.concatenate([diag, diag], axis=1))  # [128, 256]

    in_maps = []
    for h in range(NCORES):
        wqk = np.concatenate(
            [
                Wq[h * 64 : h * 64 + 64],
                Wq[512 + h * 64 : 512 + h * 64 + 64],
                Wk[h * 64 : h * 64 + 64],
                Wk[512 + h * 64 : 512 + h * 64 + 64],
                Wv[h * 128 : h * 128 + 128],
            ],
            axis=0,
        ).T  # [D, 384]
        # wpp[j, i] = Wproj[i, h*128+j] -- lhsT chunks for the partial proj
        wpp = Wproj[:, h * 128 : (h + 1) * 128].T  # [128 j, 1024 i]
        in_maps.append(
            {
                "xT": xT,
                "wqkv": np.ascontiguousarray(wqk).astype(bf),
                "wpp": np.ascontiguousarray(wpp).astype(bf),
                "cos": cos4,
                "sin": sin4,
                "diag": diag2,
            }
        )
    return in_maps


def _get_program(lam: float):
    key = round(lam, 10)
    if key not in _CACHE:
        _CACHE[key] = _build_program(lam)
    return _CACHE[key]


def kernel(x, Wq, Wk, Wv, Wproj, lambda_q1, lambda_k1, lambda_q2, lambda_k2):
    x = np.asarray(x, np.float32)
    Wq, Wk = np.asarray(Wq, np.float32), np.asarray(Wk, np.float32)
    Wv, Wproj = np.asarray(Wv, np.float32), np.asarray(Wproj, np.float32)

    lam1 = float(np.exp(np.sum(np.asarray(lambda_q1) * np.asarray(lambda_k1))))
    lam2 = float(np.exp(np.sum(np.asarray(lambda_q2) * np.asarray(lambda_k2))))
    lam = lam1 - lam2 + LAMBDA_INIT

    in_maps = _make_in_maps(x, Wq, Wk, Wv, Wproj)
    nc = _get_program(lam)

    res = run_bass_kernel_spmd(nc, in_maps, list(range(NCORES)))
    # unshard: row-parallel c_proj -> sum the 8 bf16 partial products in f32
    acc = res.results[0]["outTp"].astype(np.float32)
    for h in range(1, NCORES):
        acc += res.results[h]["outTp"].astype(np.float32)
    return np.ascontiguousarray(acc.T).reshape(1, T, D)


if __name__ == "__main__":
    rng = np.random.default_rng(0)
    ins = {
        "x": rng.standard_normal((1, T, D), np.float32),
        "Wq": (rng.standard_normal((D, D)) * 0.02).astype(np.float32),
        "Wk": (rng.standard_normal((D, D)) * 0.02).astype(np.float32),
        "Wv": (rng.standard_normal((D, D)) * 0.02).astype(np.float32),
        "Wproj": (rng.standard_normal((D, D)) * 0.02).astype(np.float32),
        "lambda_q1": (rng.standard_normal(32) * 0.1).astype(np.float32),
        "lambda_k1": (rng.standard_normal(32) * 0.1).astype(np.float32),
        "lambda_q2": (rng.standard_normal(32) * 0.1).astype(np.float32),
        "lambda_k2": (rng.standard_normal(32) * 0.1).astype(np.float32),
    }
    y = kernel(**ins)
    print("kernel output", y.shape, y.dtype, float(np.abs(y).mean()))


# revision 8
# speedup vs baseline: 1.0036x; 1.0036x over previous
"""Trainium2 Bass kernel for MixerDiffAttention (differential attention).

Sharding: tensor-parallel over the 8 (n_head//2) head groups across 8 cores
(data-parallel over B is trivial since B=1). Each core computes the QKV
projections for its head group, both differential attention branches, the
normalized combination y1 - lambda*y2, and its head's partial product with
the row-sharded c_proj. The host sums the 8 partial outputs (the unshard
step for row-parallel tensor parallelism).

v3 layout notes (per core, head h):
  - QKV projections x-stationary: out[t, (q1 q2 k1 k2 v)] per 128-row t-block,
    accumulated over 8 contraction chunks.
  - Rotary is applied to RAW q/k, then the rms_norm scale is applied to the
    rotated values (the rotation is orthogonal per (i, i+32) pair, so it
    commutes with the per-subhead scalar normalization). cos/sin tables are
    pre-expanded over the 4 subheads on the host so every rotary DVE op is a
    dense step-1 bf16 op (2x mode).
  - 1/sqrt(msq+eps) = exp(-0.5*ln(msq+eps)) computed ONCE for all 16 t-blocks
    after stage B, so ACT loads the natural_log table set once and the exp
    set once -- no per-group table thrash (Rsqrt activation is banned).
  - schedule: B0..B3 (QKV+rotary) -> rsc -> S0 C0 S1 C1 S2 C2 S3 C3, where
    S_c scales the rotated q/k group and transposes it to [c, t]: q via PE
    transposes (keeps the PE stream dense), k via the DMA xbar on the two
    then-idle HWDGE queues.
  - scores for the two branches are written into one 2-bank PSUM tile and
    exp'd in a single ACT op; the two score matmuls have K=64 at partition
    bases 0/64 so they run concurrently in the PE array (row tiling).
  - exp never overflows: q/k are rms-normalized so |score*scale| <= 8.
  - PSUM budget is exactly 8 banks: tag pp 2x2 (QKV groups / score pairs /
    proj outputs / k-stage), py 2 (PV accum + q-transposes), pd 2 (denom
    accum + q-transposes).
"""

import os
import sys

import numpy as np

for _p in ("/opt/trn_rl_repo", "/root/.axon_site/_ro/trn_rl_repo"):
    if os.path.isdir(_p) and _p not in sys.path:
        sys.path.insert(0, _p)

import ml_dtypes

import concourse.bass as bass
import concourse.mybir as mybir
import concourse.tile as tile
from concourse import bacc
from concourse.bass import ds, ts
from concourse.bass_utils import run_bass_kernel_spmd
from concourse.masks import make_identity

BF16 = mybir.dt.bfloat16
F32 = mybir.dt.float32
AF = mybir.ActivationFunctionType
ALU = mybir.AluOpType

N_HEAD = 16
D = 1024
HD = 64  # head dim
T = 2048
NCORES = 8
TB = T // 128  # 16 t-blocks
KC = D // 128  # 8 contraction chunks
NTC = T // 512  # 4 t-chunks of 512
LAMBDA_INIT = 0.8 - 0.6 * float(np.exp(-0.3 * 1))
EPS = float(np.finfo(np.float32).eps)
SCALE = 1.0 / 8.0  # 1/sqrt(64)

_CACHE = {}


def _build_program(lam: float) -> bass.Bass:
    nc = bacc.Bacc("TRN2", target_bir_lowering=False, debug=False)

    xT = nc.declare_dram_parameter("xT", [D, T], BF16, isOutput=False)
    wqkv = nc.declare_dram_parameter("wqkv", [D, 384], BF16, isOutput=False)
    wpp = nc.declare_dram_parameter("wpp", [128, D], BF16, isOutput=False)
    cos_d = nc.declare_dram_parameter("cos", [128, TB * 128], BF16, isOutput=False)
    sin_d = nc.declare_dram_parameter("sin", [128, TB * 128], BF16, isOutput=False)
    diag_d = nc.declare_dram_parameter("diag", [128, 256], BF16, isOutput=False)
    outTp = nc.declare_dram_parameter("outTp", [D, T], BF16, isOutput=True)

    with tile.TileContext(nc) as tc:
        with (
            tc.tile_pool(name="const", bufs=1) as cpool,
            tc.tile_pool(name="work", bufs=3) as wpool,
            tc.tile_pool(name="ptile", bufs=3) as ppool,
            tc.tile_pool(name="ostage", bufs=3) as opool,
            tc.tile_pool(name="psum", bufs=1, space="PSUM") as psum_pool,
        ):
            # ---- persistent SBUF tensors ----
            xT_sb = cpool.tile([128, KC, T], BF16, tag="xT")
            wqkv_sb = cpool.tile([128, KC, 384], BF16, tag="wqkv")
            wpp_sb = cpool.tile([128, KC, 128], BF16, tag="wpp")
            cos_sb = cpool.tile([128, TB, 4, 32], BF16, tag="cos")
            sin_sb = cpool.tile([128, TB, 4, 32], BF16, tag="sin")
            diag_sb = cpool.tile([128, 2, 128], BF16, tag="diag")
            ones_sb = cpool.tile([128, 128], BF16, tag="ones")
            ident_sb = cpool.tile([128, 128], BF16, tag="ident")
            qk_sb = cpool.tile([128, TB, 4, HD], BF16, tag="qk")  # raw q1 q2 k1 k2
            rot_sb = cpool.tile([128, TB, 4, HD], BF16, tag="rot")  # rotated, unscaled
            ssq_sb = cpool.tile([128, TB, 4], F32, tag="ssq")
            rsc_sb = cpool.tile([128, TB, 4], F32, tag="rsc")
            qT_sb = cpool.tile([128, T], BF16, tag="qT")  # rows 0:64 g0, 64:128 g1
            kT_sb = cpool.tile([128, T], BF16, tag="kT")
            v_sb = cpool.tile([128, TB, 128], BF16, tag="v")  # [s-part, tb, j]

            # ---- load constants ----
            # wqkv (sync) + x t-chunk 0 (gpsimd/scalar) gate the first QKV
            # matmuls; everything else streams in behind them.
            for kc in range(KC):
                nc.sync.dma_start(out=wqkv_sb[:, kc, :], in_=wqkv[ts(kc, 128), :])
            for kc in range(KC):
                eng = nc.gpsimd if kc % 2 == 0 else nc.scalar
                eng.dma_start(
                    out=xT_sb[:, kc, ts(0, 512)], in_=xT[ts(kc, 128), ts(0, 512)]
                )
            nc.scalar.dma_start(
                out=cos_sb[:].rearrange("p a b c -> p (a b c)"), in_=cos_d[:, :]
            )
            nc.scalar.dma_start(
                out=sin_sb[:].rearrange("p a b c -> p (a b c)"), in_=sin_d[:, :]
            )
            nc.gpsimd.dma_start(
                out=diag_sb[:].rearrange("p a b -> p (a b)"), in_=diag_d[:, :]
            )
            for tc_i in range(1, NTC):
                for kc in range(KC):
                    eng = (nc.sync, nc.gpsimd, nc.scalar)[kc % 3]
                    eng.dma_start(
                        out=xT_sb[:, kc, ts(tc_i, 512)],
                        in_=xT[ts(kc, 128), ts(tc_i, 512)],
                    )
            for kc in range(KC):
                nc.gpsimd.dma_start(out=wpp_sb[:, kc, :], in_=wpp[:, ts(kc, 128)])
            nc.vector.memset(ones_sb[:], 1.0)
            make_identity(nc, ident_sb[:])
            eps_sb = cpool.tile([128, 1], F32, tag="eps")
            nc.vector.memset(eps_sb[:], EPS)
            dummy_sb = cpool.tile([128, 512], BF16, tag="dummy")
            nc.vector.memset(dummy_sb[:], 0.0)

            def keepalive(n):
                """Dummy matmuls to keep the PE HAM-warm across known gaps.

                The HAM un-throttles only after ~3.4us of sustained PE busy
                and re-throttles after a ~3.4us idle window; these bursts
                warm the array during the initial DMA load phase and bridge
                the rsc/scale dependency gaps between phases.
                """
                pk = psum_pool.tile([128, 2, 512], F32, tag="pp", bufs=2, name="pk")
                for _ in range(n):
                    nc.tensor.matmul(
                        pk[:, 0, :], ones_sb[:], dummy_sb[:], start=True, stop=True
                    )

            keepalive(10)

            # ---- stage B: QKV projection + rotary for one group of 4 t-blocks
            def stage_b(c):
                for i in range(4):
                    tb = 4 * c + i
                    pqkv = psum_pool.tile(
                        [128, 2, 512], F32, tag="pp", bufs=2, name="pqkv"
                    )
                    qkv = pqkv[:].rearrange("p a b -> p (a b)")
                    for kc in range(KC):
                        nc.tensor.matmul(
                            qkv[:, 0:384],
                            xT_sb[:, kc, ts(tb, 128)],
                            wqkv_sb[:, kc, :],
                            start=(kc == 0),
                            stop=(kc == KC - 1),
                        )
                    # evacuate: v + raw qk on ACT (idle during stage B; the
                    # DVE is the stage-B bottleneck otherwise)
                    nc.scalar.copy(v_sb[:, tb, :], qkv[:, 256:384])
                    nc.scalar.copy(
                        qk_sb[:, tb].rearrange("p a b -> p (a b)"), qkv[:, 0:256]
                    )

                # sum of squares per 64-wide subhead, for the whole group
                sq = wpool.tile([128, 4, 4, HD], BF16, tag="sq")
                qkg = qk_sb[:, ds(4 * c, 4)]
                nc.vector.tensor_mul(sq[:], qkg, qkg)
                nc.vector.reduce_sum(
                    ssq_sb[:, ds(4 * c, 4)], sq[:], axis=mybir.AxisListType.X
                )

                # rotary on raw qk: out1 = n1*cos + n2*sin ; out2 = n2*cos - n1*sin
                n1 = qkg[:, :, :, 0:32]
                n2 = qkg[:, :, :, 32:64]
                cosb = cos_sb[:, ds(4 * c, 4)]
                sinb = sin_sb[:, ds(4 * c, 4)]
                rotg = rot_sb[:, ds(4 * c, 4)]
                t1 = wpool.tile([128, 4, 4, 32], BF16, tag="t1")
                t2 = wpool.tile([128, 4, 4, 32], BF16, tag="t2")
                nc.vector.tensor_mul(t1[:], n1, cosb)
                nc.vector.tensor_mul(t2[:], n2, sinb)
                nc.vector.tensor_add(rotg[:, :, :, 0:32], t1[:], t2[:])
                nc.vector.tensor_mul(t1[:], n2, cosb)
                nc.vector.tensor_mul(t2[:], n1, sinb)
                nc.vector.tensor_sub(rotg[:, :, :, 32:64], t1[:], t2[:])

            # ---- stage S: apply rms scale to one group, transpose to [c, t]
            def stage_s(c):
                qkr = wpool.tile([128, 4, 4, HD], BF16, tag="qkr")
                rscb = (
                    rsc_sb[:, ds(4 * c, 4)].unsqueeze(3).broadcast_to([128, 4, 4, HD])
                )
                nc.vector.tensor_mul(qkr[:], rot_sb[:, ds(4 * c, 4)], rscb)
                for i in range(4):
                    tb = 4 * c + i
                    qblk = qkr[:, i, 0:2, :].rearrange("p a b -> p (a b)")
                    kblk = qkr[:, i, 2:4, :].rearrange("p a b -> p (a b)")
                    # q: PE transpose (keeps the PE warm), evacuate on DVE
                    ptq = psum_pool.tile(
                        [128, 128], BF16, tag=("py" if i % 2 == 0 else "pd"),
                        name="ptq",
                    )
                    nc.tensor.transpose(ptq[:], qblk, ident_sb[:])
                    nc.vector.tensor_copy(qT_sb[:, ts(tb, 128)], ptq[:])
                    # k: DMA xbar transpose on the idle HWDGE queues
                    eng = nc.sync if i % 2 == 0 else nc.scalar
                    eng.dma_start(
                        out=kT_sb[:, ts(tb, 128)], in_=kblk, transpose=True
                    )

            # ---- stage C: differential attention + partial projection for
            # one 512-wide t-chunk ----
            def stage_c(tc_i):
                nsb = 4 * tc_i + 4  # s-blocks touching this t-chunk
                py = psum_pool.tile([128, 2, 512], F32, tag="py", name="py")
                pd = psum_pool.tile([128, 2, 512], F32, tag="pd", name="pd")
                for si in range(nsb):
                    col0 = max(0, si * 128 - tc_i * 512)
                    w = 512 - col0
                    pp = psum_pool.tile(
                        [128, 2, 512], F32, tag="pp", bufs=2, name="pp"
                    )
                    for g in range(2):
                        nc.tensor.matmul(
                            pp[:, g, col0:512],
                            kT_sb[ds(g * 64, 64), ts(si, 128)],
                            qT_sb[ds(g * 64, 64), ds(tc_i * 512 + col0, w)],
                            start=True,
                            stop=True,
                        )
                    pt = ppool.tile([128, 2, 512], BF16, tag="pt")
                    nc.scalar.activation(
                        pt[:, :, col0:512], pp[:, :, col0:512], AF.Exp, scale=SCALE
                    )
                    if col0 > 0 or si * 128 == tc_i * 512:
                        # diagonal block: zero out s > t inside it
                        nc.vector.tensor_mul(
                            pt[:, :, col0 : col0 + 128],
                            pt[:, :, col0 : col0 + 128],
                            diag_sb[:],
                        )
                    for g in range(2):
                        nc.tensor.matmul(
                            py[:, g, col0:512],
                            v_sb[:, si, :],
                            pt[:, g, col0:512],
                            start=(si == 0),
                            stop=(si == nsb - 1),
                        )
                    for g in range(2):
                        nc.tensor.matmul(
                            pd[:, g, col0:512],
                            ones_sb[:],
                            pt[:, g, col0:512],
                            start=(si == 0),
                            stop=(si == nsb - 1),
                        )

                # normalize + combine the two branches
                rec = wpool.tile([128, 2, 512], F32, tag="rec")
                nc.vector.reciprocal_approx_fast(rec[:], pd[:])
                yn = wpool.tile([128, 2, 512], BF16, tag="yn")
                nc.vector.tensor_mul(yn[:], py[:], rec[:])
                ycomb = wpool.tile([128, 512], BF16, tag="ycomb")
                nc.vector.scalar_tensor_tensor(
                    ycomb[:], yn[:, 1], -lam, yn[:, 0], ALU.mult, ALU.add
                )

                # partial projection for this t-chunk
                for icp in range(KC // 2):
                    po = psum_pool.tile([128, 2, 512], F32, tag="pp", bufs=2, name="po")
                    for j in range(2):
                        ic = icp * 2 + j
                        nc.tensor.matmul(
                            po[:, j, :],
                            wpp_sb[:, ic, :],
                            ycomb[:],
                            start=True,
                            stop=True,
                        )
                    ost = opool.tile([128, 2, 512], BF16, tag="ost")
                    if icp % 2 == 0:
                        nc.vector.tensor_copy(ost[:], po[:])
                    else:
                        nc.scalar.copy(ost[:], po[:])
                    for j in range(2):
                        ic = icp * 2 + j
                        nc.sync.dma_start(
                            out=outTp[ts(ic, 128), ts(tc_i, 512)], in_=ost[:, j]
                        )

            # ---- emission schedule ----
            for c in range(4):
                stage_b(c)
            # rsc = 1/sqrt(ssq/64 + eps) = exp(-0.5*ln(ssq/64 + eps)), one op
            # pair for all 16 t-blocks (2 ACT table loads for the whole kernel)
            lssq = wpool.tile([128, TB, 4], F32, tag="lssq")
            nc.scalar.activation(
                lssq[:], ssq_sb[:], AF.Ln, bias=eps_sb[:], scale=1.0 / HD
            )
            nc.scalar.activation(rsc_sb[:], lssq[:], AF.Exp, scale=-0.5)
            for c in range(4):
                keepalive(4)
                stage_s(c)
                stage_c(c)

    nc.compile()
    return nc


def _make_in_maps(x, Wq, Wk, Wv, Wproj):
    bf = ml_dtypes.bfloat16
    xT = np.ascontiguousarray(x[0].T).astype(bf)  # [D, T]

    # rotary tables: [tp, tb, 32] expanded over the 4 subheads -> [tp, tb, 4, 32]
    inv = 1.0 / (10000.0 ** (np.arange(0, HD, 2, dtype=np.float32) / HD))
    fr = np.outer(np.arange(T, dtype=np.float32), inv)  # [T, 32]
    cos = np.cos(fr).reshape(TB, 128, 32).transpose(1, 0, 2)  # [128, TB, 32]
    cos4 = np.broadcast_to(cos[:, :, None, :], (128, TB, 4, 32)).reshape(128, -1)
    sin = np.sin(fr).reshape(TB, 128, 32).transpose(1, 0, 2)
    sin4 = np.broadcast_to(sin[:, :, None, :], (128, TB, 4, 32)).reshape(128, -1)
    cos4, sin4 = np.ascontiguousarray(cos4).astype(bf), np.ascontiguousarray(
        sin4
    ).astype(bf)
    diag = np.triu(np.ones((128, 128), np.float32)).astype(bf)
    diag2 = np.ascontiguousarray(np.concatenate([diag, diag], axis=1))  # [128, 256]

    in_maps = []
    for h in range(NCORES):
        wqk = np.concatenate(
            [
                Wq[h * 64 : h * 64 + 64],
                Wq[512 + h * 64 : 512 + h * 64 + 64],
                Wk[h * 64 : h * 64 + 64],
                Wk[512 + h * 64 : 512 + h * 64 + 64],
                Wv[h * 128 : h * 128 + 128],
            ],
            axis=0,
        ).T  # [D, 384]
        # wpp[j, i] = Wproj[i, h*128+j] -- lhsT chunks for the partial proj
        wpp = Wproj[:, h * 128 : (h + 1) * 128].T  # [128 j, 1024 i]
        in_maps.append(
            {
                "xT": xT,
                "wqkv": np.ascontiguousarray(wqk).astype(bf),
                "wpp": np.ascontiguousarray(wpp).astype(bf),
                "cos": cos4,
                "sin": sin4,
                "diag": diag2,
            }
        )
    return in_maps


def _get_program(lam: float):
    key = round(lam, 10)
    if key not in _CACHE:
        _CACHE[key] = _build_program(lam)
    return _CACHE[key]


def kernel(x, Wq, Wk, Wv, Wproj, lambda_q1, lambda_k1, lambda_q2, lambda_k2):
    x = np.asarray(x, np.float32)
    Wq, Wk = np.asarray(Wq, np.float32), np.asarray(Wk, np.float32)
    Wv, Wproj = np.asarray(Wv, np.float32), np.asarray(Wproj, np.float32)

    lam1 = float(np.exp(np.sum(np.asarray(lambda_q1) * np.asarray(lambda_k1))))
    lam2 = float(np.exp(np.sum(np.asarray(lambda_q2) * np.asarray(lambda_k2))))
    lam = lam1 - lam2 + LAMBDA_INIT

    in_maps = _make_in_maps(x, Wq, Wk, Wv, Wproj)
    nc = _get_program(lam)

    res = run_bass_kernel_spmd(nc, in_maps, list(range(NCORES)))
    # unshard: row-parallel c_proj -> sum the 8 bf16 partial products in f32
    acc = res.results[0]["outTp"].astype(np.float32)
    for h in range(1, NCORES):
        acc += res.results[h]["outTp"].astype(np.float32)
    return np.ascontiguousarray(acc.T).reshape(1, T, D)


if __name__ == "__main__":
    rng = np.random.default_rng(0)
    ins = {
        "x": rng.standard_normal((1, T, D), np.float32),
        "Wq": (rng.standard_normal((D, D)) * 0.02).astype(np.float32),
        "Wk": (rng.standard_normal((D, D)) * 0.02).astype(np.float32),
        "Wv": (rng.standard_normal((D, D)) * 0.02).astype(np.float32),
        "Wproj": (rng.standard_normal((D, D)) * 0.02).astype(np.float32),
        "lambda_q1": (rng.standard_normal(32) * 0.1).astype(np.float32),
        "lambda_k1": (rng.standard_normal(32) * 0.1).astype(np.float32),
        "lambda_q2": (rng.standard_normal(32) * 0.1).astype(np.float32),
        "lambda_k2": (rng.standard_normal(32) * 0.1).astype(np.float32),
    }
    y = kernel(**ins)
    print("kernel output", y.shape, y.dtype, float(np.abs(y).mean()))


# revision 11
# speedup vs baseline: 1.1394x; 1.1354x over previous
"""Trainium2 Bass kernel for MixerDiffAttention (differential attention).

Sharding: tensor-parallel over the 8 (n_head//2) head groups across 8 cores
(data-parallel over B is trivial since B=1). Each core computes the QKV
projections for its head group, both differential attention branches, the
normalized combination y1 - lambda*y2, and its head's partial product with
the row-sharded c_proj. The host sums the 8 partial outputs (the unshard
step for row-parallel tensor parallelism).

v3 layout notes (per core, head h):
  - QKV projections x-stationary: out[t, (q1 q2 k1 k2 v)] per 128-row t-block,
    accumulated over 8 contraction chunks.
  - Rotary is applied to RAW q/k, then the rms_norm scale is applied to the
    rotated values (the rotation is orthogonal per (i, i+32) pair, so it
    commutes with the per-subhead scalar normalization). cos/sin tables are
    pre-expanded over the 4 subheads on the host so every rotary DVE op is a
    dense step-1 bf16 op (2x mode).
  - 1/sqrt(msq+eps) = exp(-0.5*ln(msq+eps)) computed ONCE for all 16 t-blocks
    after stage B, so ACT loads the natural_log table set once and the exp
    set once -- no per-group table thrash (Rsqrt activation is banned).
  - schedule: B0..B3 (QKV+rotary) -> rsc -> S0 C0 S1 C1 S2 C2 S3 C3, where
    S_c scales the rotated q/k group and transposes it to [c, t]: q via PE
    transposes (keeps the PE stream dense), k via the DMA xbar on the two
    then-idle HWDGE queues.
  - scores for the two branches are written into one 2-bank PSUM tile and
    exp'd in a single ACT op; the two score matmuls have K=64 at partition
    bases 0/64 so they run concurrently in the PE array (row tiling).
  - exp never overflows: q/k are rms-normalized so |score*scale| <= 8.
  - PSUM budget is exactly 8 banks: tag pp 2x2 (QKV groups / score pairs /
    proj outputs / k-stage), py 2 (PV accum + q-transposes), pd 2 (denom
    accum + q-transposes).
"""

import os
import sys

import numpy as np

for _p in ("/opt/trn_rl_repo", "/root/.axon_site/_ro/trn_rl_repo"):
    if os.path.isdir(_p) and _p not in sys.path:
        sys.path.insert(0, _p)

import ml_dtypes

import concourse.bass as bass
import concourse.mybir as mybir
import concourse.tile as tile
from concourse import bacc
from concourse.bass import ds, ts
from concourse.bass_utils import run_bass_kernel_spmd
from concourse.masks import make_identity

BF16 = mybir.dt.bfloat16
F32 = mybir.dt.float32
AF = mybir.ActivationFunctionType
ALU = mybir.AluOpType

N_HEAD = 16
D = 1024
HD = 64  # head dim
T = 2048
NCORES = 8
TB = T // 128  # 16 t-blocks
KC = D // 128  # 8 contraction chunks
NTC = T // 512  # 4 t-chunks of 512
LAMBDA_INIT = 0.8 - 0.6 * float(np.exp(-0.3 * 1))
EPS = float(np.finfo(np.float32).eps)
SCALE = 1.0 / 8.0  # 1/sqrt(64)

_CACHE = {}


def _build_program(lam: float) -> bass.Bass:
    nc = bacc.Bacc("TRN2", target_bir_lowering=False, debug=False)

    xT = nc.declare_dram_parameter("xT", [D, T], BF16, isOutput=False)
    wqkv = nc.declare_dram_parameter("wqkv", [D, 384], BF16, isOutput=False)
    wpp = nc.declare_dram_parameter("wpp", [128, D], BF16, isOutput=False)
    cos_d = nc.declare_dram_parameter("cos", [128, TB * 128], BF16, isOutput=False)
    sin_d = nc.declare_dram_parameter("sin", [128, TB * 128], BF16, isOutput=False)
    diag_d = nc.declare_dram_parameter("diag", [128, 256], BF16, isOutput=False)
    outTp = nc.declare_dram_parameter("outTp", [D, T], BF16, isOutput=True)

    with tile.TileContext(nc) as tc:
        with (
            tc.tile_pool(name="const", bufs=1) as cpool,
            tc.tile_pool(name="work", bufs=3) as wpool,
            tc.tile_pool(name="ptile", bufs=3) as ppool,
            tc.tile_pool(name="ostage", bufs=3) as opool,
            tc.tile_pool(name="psum", bufs=1, space="PSUM") as psum_pool,
        ):
            # ---- persistent SBUF tensors ----
            xT_sb = cpool.tile([128, KC, T], BF16, tag="xT")
            wqkv_sb = cpool.tile([128, KC, 384], BF16, tag="wqkv")
            wpp_sb = cpool.tile([128, KC, 128], BF16, tag="wpp")
            cos_sb = cpool.tile([128, TB, 4, 32], BF16, tag="cos")
            sin_sb = cpool.tile([128, TB, 4, 32], BF16, tag="sin")
            diag_sb = cpool.tile([128, 2, 128], BF16, tag="diag")
            ones_sb = cpool.tile([128, 128], BF16, tag="ones")
            ident_sb = cpool.tile([128, 128], BF16, tag="ident")
            qk_sb = cpool.tile([128, TB, 4, HD], BF16, tag="qk")  # raw q1 q2 k1 k2
            rot_sb = cpool.tile([128, TB, 4, HD], BF16, tag="rot")  # rotated, unscaled
            ssq_sb = cpool.tile([128, TB, 4], F32, tag="ssq")
            rsc_sb = cpool.tile([128, TB, 4], F32, tag="rsc")
            qT_sb = cpool.tile([128, T], BF16, tag="qT")  # rows 0:64 g0, 64:128 g1
            kT_sb = cpool.tile([128, T], BF16, tag="kT")
            v_sb = cpool.tile([128, TB, 128], BF16, tag="v")  # [s-part, tb, j]

            # ---- load constants ----
            # wqkv (sync) + x t-chunk 0 (gpsimd/scalar) gate the first QKV
            # matmuls; everything else streams in behind them.
            for kc in range(KC):
                nc.sync.dma_start(out=wqkv_sb[:, kc, :], in_=wqkv[ts(kc, 128), :])
            for kc in range(KC):
                eng = nc.gpsimd if kc % 2 == 0 else nc.scalar
                eng.dma_start(
                    out=xT_sb[:, kc, ts(0, 512)], in_=xT[ts(kc, 128), ts(0, 512)]
                )
            nc.scalar.dma_start(
                out=cos_sb[:].rearrange("p a b c -> p (a b c)"), in_=cos_d[:, :]
            )
            nc.scalar.dma_start(
                out=sin_sb[:].rearrange("p a b c -> p (a b c)"), in_=sin_d[:, :]
            )
            nc.gpsimd.dma_start(
                out=diag_sb[:].rearrange("p a b -> p (a b)"), in_=diag_d[:, :]
            )
            for tc_i in range(1, NTC):
                for kc in range(KC):
                    eng = (nc.sync, nc.gpsimd, nc.scalar)[kc % 3]
                    eng.dma_start(
                        out=xT_sb[:, kc, ts(tc_i, 512)],
                        in_=xT[ts(kc, 128), ts(tc_i, 512)],
                    )
            for kc in range(KC):
                nc.gpsimd.dma_start(out=wpp_sb[:, kc, :], in_=wpp[:, ts(kc, 128)])
            nc.vector.memset(ones_sb[:], 1.0)
            make_identity(nc, ident_sb[:])
            eps_sb = cpool.tile([128, 1], F32, tag="eps")
            nc.vector.memset(eps_sb[:], EPS)
            dummy_sb = cpool.tile([128, 512], BF16, tag="dummy")
            nc.vector.memset(dummy_sb[:], 0.0)

            def keepalive(n):
                """Dummy matmuls to keep the PE HAM-warm across known gaps.

                The HAM un-throttles only after ~3.4us of sustained PE busy
                and re-throttles after a ~3.4us idle window; these bursts
                warm the array during the initial DMA load phase and bridge
                the rsc/scale dependency gaps between phases.
                """
                pk = psum_pool.tile([128, 2, 512], F32, tag="pp", bufs=2, name="pk")
                for _ in range(n):
                    nc.tensor.matmul(
                        pk[:, 0, :], ones_sb[:], dummy_sb[:], start=True, stop=True
                    )

            keepalive(10)

            # ---- stage B: QKV projection + rotary for one group of 4 t-blocks
            def stage_b(c):
                for i in range(4):
                    tb = 4 * c + i
                    pqkv = psum_pool.tile(
                        [128, 2, 512], F32, tag="pp", bufs=2, name="pqkv"
                    )
                    qkv = pqkv[:].rearrange("p a b -> p (a b)")
                    for kc in range(KC):
                        nc.tensor.matmul(
                            qkv[:, 0:384],
                            xT_sb[:, kc, ts(tb, 128)],
                            wqkv_sb[:, kc, :],
                            start=(kc == 0),
                            stop=(kc == KC - 1),
                        )
                    # evacuate: v + raw qk on ACT (idle during stage B; the
                    # DVE is the stage-B bottleneck otherwise)
                    nc.scalar.copy(v_sb[:, tb, :], qkv[:, 256:384])
                    nc.scalar.copy(
                        qk_sb[:, tb].rearrange("p a b -> p (a b)"), qkv[:, 0:256]
                    )

                # sum of squares per 64-wide subhead, for the whole group
                sq = wpool.tile([128, 4, 4, HD], BF16, tag="sq")
                qkg = qk_sb[:, ds(4 * c, 4)]
                nc.vector.tensor_mul(sq[:], qkg, qkg)
                nc.vector.reduce_sum(
                    ssq_sb[:, ds(4 * c, 4)], sq[:], axis=mybir.AxisListType.X
                )

                # rotary on raw qk: out1 = n1*cos + n2*sin ; out2 = n2*cos - n1*sin
                n1 = qkg[:, :, :, 0:32]
                n2 = qkg[:, :, :, 32:64]
                cosb = cos_sb[:, ds(4 * c, 4)]
                sinb = sin_sb[:, ds(4 * c, 4)]
                rotg = rot_sb[:, ds(4 * c, 4)]
                t1 = wpool.tile([128, 4, 4, 32], BF16, tag="t1")
                t2 = wpool.tile([128, 4, 4, 32], BF16, tag="t2")
                nc.vector.tensor_mul(t1[:], n1, cosb)
                nc.vector.tensor_mul(t2[:], n2, sinb)
                nc.vector.tensor_add(rotg[:, :, :, 0:32], t1[:], t2[:])
                nc.vector.tensor_mul(t1[:], n2, cosb)
                nc.vector.tensor_mul(t2[:], n1, sinb)
                nc.vector.tensor_sub(rotg[:, :, :, 32:64], t1[:], t2[:])

            # ---- stage S: apply rms scale to one group, transpose to [c, t]
            def stage_s(c):
                qkr = wpool.tile([128, 4, 4, HD], BF16, tag="qkr")
                rscb = (
                    rsc_sb[:, ds(4 * c, 4)].unsqueeze(3).broadcast_to([128, 4, 4, HD])
                )
                nc.vector.tensor_mul(qkr[:], rot_sb[:, ds(4 * c, 4)], rscb)
                for i in range(4):
                    tb = 4 * c + i
                    qblk = qkr[:, i, 0:2, :].rearrange("p a b -> p (a b)")
                    kblk = qkr[:, i, 2:4, :].rearrange("p a b -> p (a b)")
                    # q: PE transpose (keeps the PE warm), evacuate on DVE
                    ptq = psum_pool.tile(
                        [128, 128], BF16, tag="pp", bufs=2, name="ptq"
                    )
                    nc.tensor.transpose(ptq[:], qblk, ident_sb[:])
                    nc.vector.tensor_copy(qT_sb[:, ts(tb, 128)], ptq[:])
                    # k: DMA xbar transpose on the idle HWDGE queues
                    eng = nc.sync if i % 2 == 0 else nc.scalar
                    eng.dma_start(
                        out=kT_sb[:, ts(tb, 128)], in_=kblk, transpose=True
                    )

            # ---- stage C: differential attention for one 512-wide t-chunk,
            # software-pipelined: SC(si+1) is emitted BEFORE exp(si)/PVD(si)
            # so the PE queue never blocks on the ACT exp with an empty
            # pipeline (the PVD(si) stall overlaps SC(si+1) and the
            # previous-chunk projection matmuls) ----
            def stage_proj(tc_i, ycomb):
                for icp in range(KC // 2):
                    po = psum_pool.tile([128, 2, 512], F32, tag="pp", bufs=2, name="po")
                    for j in range(2):
                        ic = icp * 2 + j
                        nc.tensor.matmul(
                            po[:, j, :],
                            wpp_sb[:, ic, :],
                            ycomb[:],
                            start=True,
                            stop=True,
                        )
                    ost = opool.tile([128, 2, 512], BF16, tag="ost")
                    if icp % 2 == 0:
                        nc.vector.tensor_copy(ost[:], po[:])
                    else:
                        nc.scalar.copy(ost[:], po[:])
                    for j in range(2):
                        ic = icp * 2 + j
                        nc.sync.dma_start(
                            out=outTp[ts(ic, 128), ts(tc_i, 512)], in_=ost[:, j]
                        )

            def stage_c(tc_i, proj_prev=None):
                nsb = 4 * tc_i + 4  # s-blocks touching this t-chunk
                py = psum_pool.tile([128, 2, 512], F32, tag="py", name="py")
                pd = psum_pool.tile([128, 2, 512], F32, tag="pd", name="pd")

                def col0_of(si):
                    return max(0, si * 128 - tc_i * 512)

                def do_sc(si):
                    col0 = col0_of(si)
                    w = 512 - col0
                    pp = psum_pool.tile(
                        [128, 2, 512], F32, tag="pp", bufs=2, name="pp"
                    )
                    for g in range(2):
                        nc.tensor.matmul(
                            pp[:, g, col0:512],
                            kT_sb[ds(g * 64, 64), ts(si, 128)],
                            qT_sb[ds(g * 64, 64), ds(tc_i * 512 + col0, w)],
                            start=True,
                            stop=True,
                        )
                    return pp

                pps = {0: do_sc(0)}
                if nsb > 1:
                    pps[1] = do_sc(1)
                if proj_prev is not None:
                    # previous chunk's projection fills the first exp latency
                    stage_proj(*proj_prev)
                for si in range(nsb):
                    col0 = col0_of(si)
                    pp = pps.pop(si)
                    pt = ppool.tile([128, 2, 512], BF16, tag="pt")
                    nc.scalar.activation(
                        pt[:, :, col0:512], pp[:, :, col0:512], AF.Exp, scale=SCALE
                    )
                    if col0 > 0 or si * 128 == tc_i * 512:
                        # diagonal block: zero out s > t inside it
                        nc.vector.tensor_mul(
                            pt[:, :, col0 : col0 + 128],
                            pt[:, :, col0 : col0 + 128],
                            diag_sb[:],
                        )
                    for g in range(2):
                        nc.tensor.matmul(
                            py[:, g, col0:512],
                            v_sb[:, si, :],
                            pt[:, g, col0:512],
                            start=(si == 0),
                            stop=(si == nsb - 1),
                        )
                    for g in range(2):
                        nc.tensor.matmul(
                            pd[:, g, col0:512],
                            ones_sb[:],
                            pt[:, g, col0:512],
                            start=(si == 0),
                            stop=(si == nsb - 1),
                        )
                    if si + 2 < nsb:
                        pps[si + 2] = do_sc(si + 2)

                # normalize + combine the two branches
                rec = wpool.tile([128, 2, 512], F32, tag="rec")
                nc.vector.reciprocal_approx_fast(rec[:], pd[:])
                yn = wpool.tile([128, 2, 512], BF16, tag="yn")
                nc.vector.tensor_mul(yn[:], py[:], rec[:])
                ycomb = wpool.tile([128, 512], BF16, tag="ycomb")
                nc.vector.scalar_tensor_tensor(
                    ycomb[:], yn[:, 1], -lam, yn[:, 0], ALU.mult, ALU.add
                )
                return ycomb

            # ---- emission schedule ----
            for c in range(4):
                stage_b(c)
            # rsc = 1/sqrt(ssq/64 + eps) = exp(-0.5*ln(ssq/64 + eps)), one op
            # pair for all 16 t-blocks (2 ACT table loads for the whole kernel)
            lssq = wpool.tile([128, TB, 4], F32, tag="lssq")
            nc.scalar.activation(
                lssq[:], ssq_sb[:], AF.Ln, bias=eps_sb[:], scale=1.0 / HD
            )
            nc.scalar.activation(rsc_sb[:], lssq[:], AF.Exp, scale=-0.5)
            proj_prev = None
            for c in range(4):
                keepalive(3)
                stage_s(c)
                ycomb = stage_c(c, proj_prev)
                proj_prev = (c, ycomb)
            stage_proj(*proj_prev)

    nc.compile()
    return nc


def _make_in_maps(x, Wq, Wk, Wv, Wproj):
    bf = ml_dtypes.bfloat16
    xT = np.ascontiguousarray(x[0].T).astype(bf)  # [D, T]

    # rotary tables: [tp, tb, 32] expanded over the 4 subheads -> [tp, tb, 4, 32]
    inv = 1.0 / (10000.0 ** (np.arange(0, HD, 2, dtype=np.float32) / HD))
    fr = np.outer(np.arange(T, dtype=np.float32), inv)  # [T, 32]
    cos = np.cos(fr).reshape(TB, 128, 32).transpose(1, 0, 2)  # [128, TB, 32]
    cos4 = np.broadcast_to(cos[:, :, None, :], (128, TB, 4, 32)).reshape(128, -1)
    sin = np.sin(fr).reshape(TB, 128, 32).transpose(1, 0, 2)
    sin4 = np.broadcast_to(sin[:, :, None, :], (128, TB, 4, 32)).reshape(128, -1)
    cos4, sin4 = np.ascontiguousarray(cos4).astype(bf), np.ascontiguousarray(
        sin4
    ).astype(bf)
    diag = np.triu(np.ones((128, 128), np.float32)).astype(bf)
    diag2 = np.ascontiguousarray(np.concatenate([diag, diag], axis=1))  # [128, 256]

    in_maps = []
    for h in range(NCORES):
        wqk = np.concatenate(
            [
                Wq[h * 64 : h * 64 + 64],
                Wq[512 + h * 64 : 512 + h * 64 + 64],
                Wk[h * 64 : h * 64 + 64],
                Wk[512 + h * 64 : 512 + h * 64 + 64],
                Wv[h * 128 : h * 128 + 128],
            ],
            axis=0,
        ).T  # [D, 384]
        # wpp[j, i] = Wproj[i, h*128+j] -- lhsT chunks for the partial proj
        wpp = Wproj[:, h * 128 : (h + 1) * 128].T  # [128 j, 1024 i]
        in_maps.append(
            {
                "xT": xT,
                "wqkv": np.ascontiguousarray(wqk).astype(bf),
                "wpp": np.ascontiguousarray(wpp).astype(bf),
                "cos": cos4,
                "sin": sin4,
                "diag": diag2,
            }
        )
    return in_maps


def _get_program(lam: float):
    key = round(lam, 10)
    if key not in _CACHE:
        _CACHE[key] = _build_program(lam)
    return _CACHE[key]


def kernel(x, Wq, Wk, Wv, Wproj, lambda_q1, lambda_k1, lambda_q2, lambda_k2):
    x = np.asarray(x, np.float32)
    Wq, Wk = np.asarray(Wq, np.float32), np.asarray(Wk, np.float32)
    Wv, Wproj = np.asarray(Wv, np.float32), np.asarray(Wproj, np.float32)

    lam1 = float(np.exp(np.sum(np.asarray(lambda_q1) * np.asarray(lambda_k1))))
    lam2 = float(np.exp(np.sum(np.asarray(lambda_q2) * np.asarray(lambda_k2))))
    lam = lam1 - lam2 + LAMBDA_INIT

    in_maps = _make_in_maps(x, Wq, Wk, Wv, Wproj)
    nc = _get_program(lam)

    res = run_bass_kernel_spmd(nc, in_maps, list(range(NCORES)))
    # unshard: row-parallel c_proj -> sum the 8 bf16 partial products in f32
    acc = res.results[0]["outTp"].astype(np.float32)
    for h in range(1, NCORES):
        acc += res.results[h]["outTp"].astype(np.float32)
    return np.ascontiguousarray(acc.T).reshape(1, T, D)


if __name__ == "__main__":
    rng = np.random.default_rng(0)
    ins = {
        "x": rng.standard_normal((1, T, D), np.float32),
        "Wq": (rng.standard_normal((D, D)) * 0.02).astype(np.float32),
        "Wk": (rng.standard_normal((D, D)) * 0.02).astype(np.float32),
        "Wv": (rng.standard_normal((D, D)) * 0.02).astype(np.float32),
        "Wproj": (rng.standard_normal((D, D)) * 0.02).astype(np.float32),
        "lambda_q1": (rng.standard_normal(32) * 0.1).astype(np.float32),
        "lambda_k1": (rng.standard_normal(32) * 0.1).astype(np.float32),
        "lambda_q2": (rng.standard_normal(32) * 0.1).astype(np.float32),
        "lambda_k2": (rng.standard_normal(32) * 0.1).astype(np.float32),
    }
    y = kernel(**ins)
    print("kernel output", y.shape, y.dtype, float(np.abs(y).mean()))


# revision 15
# speedup vs baseline: 1.1592x; 1.0174x over previous
"""Trainium2 Bass kernel for MixerDiffAttention (differential attention).

Sharding: tensor-parallel over the 8 (n_head//2) head groups across 8 cores
(data-parallel over B is trivial since B=1). Each core computes the QKV
projections for its head group, both differential attention branches, the
normalized combination y1 - lambda*y2, and its head's partial product with
the row-sharded c_proj. The host sums the 8 partial outputs (the unshard
step for row-parallel tensor parallelism).

v3 layout notes (per core, head h):
  - QKV projections x-stationary: out[t, (q1 q2 k1 k2 v)] per 128-row t-block,
    accumulated over 8 contraction chunks.
  - Rotary is applied to RAW q/k, then the rms_norm scale is applied to the
    rotated values (the rotation is orthogonal per (i, i+32) pair, so it
    commutes with the per-subhead scalar normalization). cos/sin tables are
    pre-expanded over the 4 subheads on the host so every rotary DVE op is a
    dense step-1 bf16 op (2x mode).
  - 1/sqrt(msq+eps) = exp(-0.5*ln(msq+eps)) computed ONCE for all 16 t-blocks
    after stage B, so ACT loads the natural_log table set once and the exp
    set once -- no per-group table thrash (Rsqrt activation is banned).
  - schedule: B0..B3 (QKV+rotary) -> rsc -> S0 C0 S1 C1 S2 C2 S3 C3, where
    S_c scales the rotated q/k group and transposes it to [c, t]: q via PE
    transposes (keeps the PE stream dense), k via the DMA xbar on the two
    then-idle HWDGE queues.
  - scores for the two branches are written into one 2-bank PSUM tile and
    exp'd in a single ACT op; the two score matmuls have K=64 at partition
    bases 0/64 so they run concurrently in the PE array (row tiling).
  - exp never overflows: q/k are rms-normalized so |score*scale| <= 8.
  - PSUM budget is exactly 8 banks: tag pp 2x2 (QKV groups / score pairs /
    proj outputs / k-stage), py 2 (PV accum + q-transposes), pd 2 (denom
    accum + q-transposes).
"""

import os
import sys

import numpy as np

for _p in ("/opt/trn_rl_repo", "/root/.axon_site/_ro/trn_rl_repo"):
    if os.path.isdir(_p) and _p not in sys.path:
        sys.path.insert(0, _p)

import ml_dtypes

import concourse.bass as bass
import concourse.mybir as mybir
import concourse.tile as tile
from concourse import bacc
from concourse.bass import ds, ts
from concourse.bass_utils import run_bass_kernel_spmd
from concourse.masks import make_identity

BF16 = mybir.dt.bfloat16
F32 = mybir.dt.float32
AF = mybir.ActivationFunctionType
ALU = mybir.AluOpType

N_HEAD = 16
D = 1024
HD = 64  # head dim
T = 2048
NCORES = 8
TB = T // 128  # 16 t-blocks
KC = D // 128  # 8 contraction chunks
NTC = T // 512  # 4 t-chunks of 512
LAMBDA_INIT = 0.8 - 0.6 * float(np.exp(-0.3 * 1))
EPS = float(np.finfo(np.float32).eps)
SCALE = 1.0 / 8.0  # 1/sqrt(64)

_CACHE = {}


def _build_program(lam: float) -> bass.Bass:
    nc = bacc.Bacc("TRN2", target_bir_lowering=False, debug=False)

    xT = nc.declare_dram_parameter("xT", [D, T], BF16, isOutput=False)
    wqkv = nc.declare_dram_parameter("wqkv", [D, 384], BF16, isOutput=False)
    wpp = nc.declare_dram_parameter("wpp", [128, D], BF16, isOutput=False)
    cos_d = nc.declare_dram_parameter("cos", [128, TB * 128], BF16, isOutput=False)
    sin_d = nc.declare_dram_parameter("sin", [128, TB * 128], BF16, isOutput=False)
    diag_d = nc.declare_dram_parameter("diag", [128, 256], BF16, isOutput=False)
    outTp = nc.declare_dram_parameter("outTp", [D, T], BF16, isOutput=True)

    with tile.TileContext(nc) as tc:
        with (
            tc.tile_pool(name="const", bufs=1) as cpool,
            tc.tile_pool(name="work", bufs=3) as wpool,
            tc.tile_pool(name="ptile", bufs=3) as ppool,
            tc.tile_pool(name="ostage", bufs=3) as opool,
            tc.tile_pool(name="psum", bufs=1, space="PSUM") as psum_pool,
        ):
            # ---- persistent SBUF tensors ----
            xT_sb = cpool.tile([128, KC, T], BF16, tag="xT")
            wqkv_sb = cpool.tile([128, KC, 384], BF16, tag="wqkv")
            wpp_sb = cpool.tile([128, KC, 128], BF16, tag="wpp")
            cos_sb = cpool.tile([128, TB, 4, 32], BF16, tag="cos")
            sin_sb = cpool.tile([128, TB, 4, 32], BF16, tag="sin")
            diag_sb = cpool.tile([128, 2, 128], BF16, tag="diag")
            ones_sb = cpool.tile([128, 128], BF16, tag="ones")
            ident_sb = cpool.tile([128, 128], BF16, tag="ident")
            qk_sb = cpool.tile([128, TB, 4, HD], BF16, tag="qk")  # raw q1 q2 k1 k2
            rot_sb = cpool.tile([128, TB, 4, HD], BF16, tag="rot")  # rotated, unscaled
            ssq_sb = cpool.tile([128, TB, 4], F32, tag="ssq")
            rsc_sb = cpool.tile([128, TB, 4], F32, tag="rsc")
            qT_sb = cpool.tile([128, T], BF16, tag="qT")  # rows 0:64 g0, 64:128 g1
            kT_sb = cpool.tile([128, T], BF16, tag="kT")
            v_sb = cpool.tile([128, TB, 128], BF16, tag="v")  # [s-part, tb, j]

            # ---- load constants ----
            # wqkv (sync) + x t-chunk 0 (gpsimd/scalar) gate the first QKV
            # matmuls; everything else streams in behind them.
            for kc in range(KC):
                nc.sync.dma_start(out=wqkv_sb[:, kc, :], in_=wqkv[ts(kc, 128), :])
            for kc in range(KC):
                eng = nc.gpsimd if kc % 2 == 0 else nc.scalar
                eng.dma_start(
                    out=xT_sb[:, kc, ts(0, 512)], in_=xT[ts(kc, 128), ts(0, 512)]
                )
            nc.scalar.dma_start(
                out=cos_sb[:].rearrange("p a b c -> p (a b c)"), in_=cos_d[:, :]
            )
            nc.scalar.dma_start(
                out=sin_sb[:].rearrange("p a b c -> p (a b c)"), in_=sin_d[:, :]
            )
            nc.gpsimd.dma_start(
                out=diag_sb[:].rearrange("p a b -> p (a b)"), in_=diag_d[:, :]
            )
            for tc_i in range(1, NTC):
                for kc in range(KC):
                    eng = (nc.sync, nc.gpsimd, nc.scalar)[kc % 3]
                    eng.dma_start(
                        out=xT_sb[:, kc, ts(tc_i, 512)],
                        in_=xT[ts(kc, 128), ts(tc_i, 512)],
                    )
            for kc in range(KC):
                nc.gpsimd.dma_start(out=wpp_sb[:, kc, :], in_=wpp[:, ts(kc, 128)])
            nc.vector.memset(ones_sb[:], 1.0)
            make_identity(nc, ident_sb[:])
            eps_sb = cpool.tile([128, 1], F32, tag="eps")
            nc.vector.memset(eps_sb[:], EPS)
            dummy_sb = cpool.tile([128, 512], BF16, tag="dummy")
            nc.vector.memset(dummy_sb[:], 0.0)

            def keepalive(n):
                """Dummy matmuls to keep the PE HAM-warm across known gaps.

                The HAM un-throttles only after ~3.4us of sustained PE busy
                and re-throttles after a ~3.4us idle window; these bursts
                warm the array during the initial DMA load phase and bridge
                the rsc/scale dependency gaps between phases.
                """
                pk = psum_pool.tile([128, 2, 512], F32, tag="pp", bufs=2, name="pk")
                for _ in range(n):
                    nc.tensor.matmul(
                        pk[:, 0, :], ones_sb[:], dummy_sb[:], start=True, stop=True
                    )

            keepalive(10)

            # ---- stage B: QKV projection + rotary for one group of 4 t-blocks
            def stage_b(c):
                for i in range(4):
                    tb = 4 * c + i
                    pqkv = psum_pool.tile(
                        [128, 2, 512], F32, tag="pp", bufs=2, name="pqkv"
                    )
                    qkv = pqkv[:].rearrange("p a b -> p (a b)")
                    for kc in range(KC):
                        nc.tensor.matmul(
                            qkv[:, 0:384],
                            xT_sb[:, kc, ts(tb, 128)],
                            wqkv_sb[:, kc, :],
                            start=(kc == 0),
                            stop=(kc == KC - 1),
                        )
                    # evacuate: v + raw qk on ACT (idle during stage B; the
                    # DVE is the stage-B bottleneck otherwise)
                    nc.scalar.copy(v_sb[:, tb, :], qkv[:, 256:384])
                    nc.scalar.copy(
                        qk_sb[:, tb].rearrange("p a b -> p (a b)"), qkv[:, 0:256]
                    )

                # sum of squares per 64-wide subhead, for the whole group
                sq = wpool.tile([128, 4, 4, HD], BF16, tag="sq")
                qkg = qk_sb[:, ds(4 * c, 4)]
                nc.vector.tensor_mul(sq[:], qkg, qkg)
                nc.vector.reduce_sum(
                    ssq_sb[:, ds(4 * c, 4)], sq[:], axis=mybir.AxisListType.X
                )

                # rotary on raw qk: out1 = n1*cos + n2*sin ; out2 = n2*cos - n1*sin
                n1 = qkg[:, :, :, 0:32]
                n2 = qkg[:, :, :, 32:64]
                cosb = cos_sb[:, ds(4 * c, 4)]
                sinb = sin_sb[:, ds(4 * c, 4)]
                rotg = rot_sb[:, ds(4 * c, 4)]
                t1 = wpool.tile([128, 4, 4, 32], BF16, tag="t1")
                t2 = wpool.tile([128, 4, 4, 32], BF16, tag="t2")
                nc.vector.tensor_mul(t1[:], n1, cosb)
                nc.vector.tensor_mul(t2[:], n2, sinb)
                nc.vector.tensor_add(rotg[:, :, :, 0:32], t1[:], t2[:])
                nc.vector.tensor_mul(t1[:], n2, cosb)
                nc.vector.tensor_mul(t2[:], n1, sinb)
                nc.vector.tensor_sub(rotg[:, :, :, 32:64], t1[:], t2[:])

            # ---- stage S: apply rms scale to one group, transpose to [c, t]
            def stage_s(c):
                qkr = wpool.tile([128, 4, 4, HD], BF16, tag="qkr")
                rscb = (
                    rsc_sb[:, ds(4 * c, 4)].unsqueeze(3).broadcast_to([128, 4, 4, HD])
                )
                nc.vector.tensor_mul(qkr[:], rot_sb[:, ds(4 * c, 4)], rscb)
                for i in range(4):
                    tb = 4 * c + i
                    qblk = qkr[:, i, 0:2, :].rearrange("p a b -> p (a b)")
                    kblk = qkr[:, i, 2:4, :].rearrange("p a b -> p (a b)")
                    # q: PE transpose (keeps the PE warm), evacuate on DVE
                    ptq = psum_pool.tile(
                        [128, 128], BF16, tag="pp", bufs=2, name="ptq"
                    )
                    nc.tensor.transpose(ptq[:], qblk, ident_sb[:])
                    nc.vector.tensor_copy(qT_sb[:, ts(tb, 128)], ptq[:])
                    # k: DMA xbar transpose on the idle HWDGE queues
                    eng = nc.sync if i % 2 == 0 else nc.scalar
                    eng.dma_start(
                        out=kT_sb[:, ts(tb, 128)], in_=kblk, transpose=True
                    )

            # ---- stage C: differential attention for one 512-wide t-chunk,
            # software-pipelined: SC(si+1) is emitted BEFORE exp(si)/PVD(si)
            # so the PE queue never blocks on the ACT exp with an empty
            # pipeline (the PVD(si) stall overlaps SC(si+1) and the
            # previous-chunk projection matmuls) ----
            def proj_step(tc_i, ycomb, icp):
                po = psum_pool.tile([128, 2, 512], F32, tag="pp", bufs=2, name="po")
                for j in range(2):
                    ic = icp * 2 + j
                    nc.tensor.matmul(
                        po[:, j, :],
                        wpp_sb[:, ic, :],
                        ycomb[:],
                        start=True,
                        stop=True,
                    )
                ost = opool.tile([128, 2, 512], BF16, tag="ost")
                if icp % 2 == 0:
                    nc.vector.tensor_copy(ost[:], po[:])
                else:
                    nc.scalar.copy(ost[:], po[:])
                for j in range(2):
                    ic = icp * 2 + j
                    nc.sync.dma_start(
                        out=outTp[ts(ic, 128), ts(tc_i, 512)], in_=ost[:, j]
                    )

            def stage_proj(tc_i, ycomb):
                for icp in range(KC // 2):
                    proj_step(tc_i, ycomb, icp)

            def stage_c(tc_i, proj_prev=None):
                nsb = 4 * tc_i + 4  # s-blocks touching this t-chunk
                py = psum_pool.tile([128, 2, 512], F32, tag="py", name="py")
                pd = psum_pool.tile([128, 2, 512], F32, tag="pd", name="pd")

                def col0_of(si):
                    return max(0, si * 128 - tc_i * 512)

                def do_sc(si):
                    col0 = col0_of(si)
                    w = 512 - col0
                    pp = psum_pool.tile(
                        [128, 2, 512], F32, tag="pp", bufs=2, name="pp"
                    )
                    for g in range(2):
                        nc.tensor.matmul(
                            pp[:, g, col0:512],
                            kT_sb[ds(g * 64, 64), ts(si, 128)],
                            qT_sb[ds(g * 64, 64), ds(tc_i * 512 + col0, w)],
                            start=True,
                            stop=True,
                        )
                    return pp

                # spread the previous chunk's 4 projection steps through the
                # si loop: the extra ~0.4us of PE work per marked si keeps the
                # PE fully busy past the exp latency, so the HAM never sees a
                # micro-idle and the clock stays at 2.4 GHz
                proj_at = {}
                if proj_prev is not None:
                    for icp in range(KC // 2):
                        proj_at[(icp * nsb) // (KC // 2)] = icp

                pps = {0: do_sc(0)}
                if nsb > 1:
                    pps[1] = do_sc(1)
                for si in range(nsb):
                    col0 = col0_of(si)
                    pp = pps.pop(si)
                    pt = ppool.tile([128, 2, 512], BF16, tag="pt")
                    nc.scalar.activation(
                        pt[:, :, col0:512], pp[:, :, col0:512], AF.Exp, scale=SCALE
                    )
                    if col0 > 0 or si * 128 == tc_i * 512:
                        # diagonal block: zero out s > t inside it
                        nc.vector.tensor_mul(
                            pt[:, :, col0 : col0 + 128],
                            pt[:, :, col0 : col0 + 128],
                            diag_sb[:],
                        )
                    for g in range(2):
                        nc.tensor.matmul(
                            py[:, g, col0:512],
                            v_sb[:, si, :],
                            pt[:, g, col0:512],
                            start=(si == 0),
                            stop=(si == nsb - 1),
                        )
                    for g in range(2):
                        nc.tensor.matmul(
                            pd[:, g, col0:512],
                            ones_sb[:],
                            pt[:, g, col0:512],
                            start=(si == 0),
                            stop=(si == nsb - 1),
                        )
                    if si in proj_at:
                        proj_step(proj_prev[0], proj_prev[1], proj_at[si])
                    if si + 2 < nsb:
                        pps[si + 2] = do_sc(si + 2)

                # normalize + combine the two branches
                rec = wpool.tile([128, 2, 512], F32, tag="rec")
                nc.vector.reciprocal_approx_fast(rec[:], pd[:])
                yn = wpool.tile([128, 2, 512], BF16, tag="yn")
                nc.vector.tensor_mul(yn[:], py[:], rec[:])
                ycomb = wpool.tile([128, 512], BF16, tag="ycomb")
                nc.vector.scalar_tensor_tensor(
                    ycomb[:], yn[:, 1], -lam, yn[:, 0], ALU.mult, ALU.add
                )
                return ycomb

            # ---- emission schedule ----
            for c in range(4):
                stage_b(c)
            # rsc = 1/sqrt(ssq/64 + eps) = exp(-0.5*ln(ssq/64 + eps)), one op
            # pair for all 16 t-blocks (2 ACT table loads for the whole kernel)
            lssq = wpool.tile([128, TB, 4], F32, tag="lssq")
            nc.scalar.activation(
                lssq[:], ssq_sb[:], AF.Ln, bias=eps_sb[:], scale=1.0 / HD
            )
            nc.scalar.activation(rsc_sb[:], lssq[:], AF.Exp, scale=-0.5)
            proj_prev = None
            for c in range(4):
                keepalive(10 if c == 0 else 3)
                stage_s(c)
                ycomb = stage_c(c, proj_prev)
                proj_prev = (c, ycomb)
            stage_proj(*proj_prev)

    nc.compile()
    return nc


def _make_in_maps(x, Wq, Wk, Wv, Wproj):
    bf = ml_dtypes.bfloat16
    xT = np.ascontiguousarray(x[0].T).astype(bf)  # [D, T]

    # rotary tables: [tp, tb, 32] expanded over the 4 subheads -> [tp, tb, 4, 32]
    inv = 1.0 / (10000.0 ** (np.arange(0, HD, 2, dtype=np.float32) / HD))
    fr = np.outer(np.arange(T, dtype=np.float32), inv)  # [T, 32]
    cos = np.cos(fr).reshape(TB, 128, 32).transpose(1, 0, 2)  # [128, TB, 32]
    cos4 = np.broadcast_to(cos[:, :, None, :], (128, TB, 4, 32)).reshape(128, -1)
    sin = np.sin(fr).reshape(TB, 128, 32).transpose(1, 0, 2)
    sin4 = np.broadcast_to(sin[:, :, None, :], (128, TB, 4, 32)).reshape(128, -1)
    cos4, sin4 = np.ascontiguousarray(cos4).astype(bf), np.ascontiguousarray(
        sin4
    ).astype(bf)
    diag = np.triu(np.ones((128, 128), np.float32)).astype(bf)
    diag2 = np.ascontiguousarray(np.concatenate([diag, diag], axis=1))  # [128, 256]

    in_maps = []
    for h in range(NCORES):
        wqk = np.concatenate(
            [
                Wq[h * 64 : h * 64 + 64],
                Wq[512 + h * 64 : 512 + h * 64 + 64],
                Wk[h * 64 : h * 64 + 64],
                Wk[512 + h * 64 : 512 + h * 64 + 64],
                Wv[h * 128 : h * 128 + 128],
            ],
            axis=0,
        ).T  # [D, 384]
        # wpp[j, i] = Wproj[i, h*128+j] -- lhsT chunks for the partial proj
        wpp = Wproj[:, h * 128 : (h + 1) * 128].T  # [128 j, 1024 i]
        in_maps.append(
            {
                "xT": xT,
                "wqkv": np.ascontiguousarray(wqk).astype(bf),
                "wpp": np.ascontiguousarray(wpp).astype(bf),
                "cos": cos4,
                "sin": sin4,
                "diag": diag2,
            }
        )
    return in_maps


def _get_program(lam: float):
    key = round(lam, 10)
    if key not in _CACHE:
        _CACHE[key] = _build_program(lam)
    return _CACHE[key]


def kernel(x, Wq, Wk, Wv, Wproj, lambda_q1, lambda_k1, lambda_q2, lambda_k2):
    x = np.asarray(x, np.float32)
    Wq, Wk = np.asarray(Wq, np.float32), np.asarray(Wk, np.float32)
    Wv, Wproj = np.asarray(Wv, np.float32), np.asarray(Wproj, np.float32)

    lam1 = float(np.exp(np.sum(np.asarray(lambda_q1) * np.asarray(lambda_k1))))
    lam2 = float(np.exp(np.sum(np.asarray(lambda_q2) * np.asarray(lambda_k2))))
    lam = lam1 - lam2 + LAMBDA_INIT

    in_maps = _make_in_maps(x, Wq, Wk, Wv, Wproj)
    nc = _get_program(lam)

    res = run_bass_kernel_spmd(nc, in_maps, list(range(NCORES)))
    # unshard: row-parallel c_proj -> sum the 8 bf16 partial products in f32
    acc = res.results[0]["outTp"].astype(np.float32)
    for h in range(1, NCORES):
        acc += res.results[h]["outTp"].astype(np.float32)
    return np.ascontiguousarray(acc.T).reshape(1, T, D)


if __name__ == "__main__":
    rng = np.random.default_rng(0)
    ins = {
        "x": rng.standard_normal((1, T, D), np.float32),
        "Wq": (rng.standard_normal((D, D)) * 0.02).astype(np.float32),
        "Wk": (rng.standard_normal((D, D)) * 0.02).astype(np.float32),
        "Wv": (rng.standard_normal((D, D)) * 0.02).astype(np.float32),
        "Wproj": (rng.standard_normal((D, D)) * 0.02).astype(np.float32),
        "lambda_q1": (rng.standard_normal(32) * 0.1).astype(np.float32),
        "lambda_k1": (rng.standard_normal(32) * 0.1).astype(np.float32),
        "lambda_q2": (rng.standard_normal(32) * 0.1).astype(np.float32),
        "lambda_k2": (rng.standard_normal(32) * 0.1).astype(np.float32),
    }
    y = kernel(**ins)
    print("kernel output", y.shape, y.dtype, float(np.abs(y).mean()))
